# revision 1
# baseline (speedup 1.0000x reference)
"""Multi-head self-attention 2D (dense transformer) Bass kernel for Trainium2.

Problem: x [4, 512, 48, 48] fp32; qkv_w [1536, 512]; proj_w [512, 512].
  qkv 1x1-conv -> per-head attention (8 heads, head_dim 64) over N=2304
  spatial positions -> output projection.

Sharding (8 cores): core i handles batch b = i//2 and query half i%2
  (nq = 1152 queries). Each core computes K/V for the full batch image
  (keys/values need all N positions) and the final projection for its
  query columns, so per-core outputs are disjoint slices of the full
  output -- no collectives, gather on host.

Per-core pipeline (matmul operands fp16, PSUM fp32):
  1. q = Wq @ xq, k = Wk @ xk, vT = xk^T @ WvT  (cast fp16)
  2. attention per head-pair with transposed scores S^T = k_h^T q_h
     (row-packed PE pairs), wide exp on ScalarE (scale=1/8 folded in),
     DVE accumulates softmax denominators, AV matmuls accumulate in
     PSUM.  Softmax max-subtraction is skipped: scores*scale ~ N(0,1)
     so exp stays in range.
  3. denominators via ones-matmul, reciprocal, broadcast via K=1
     outer-product matmuls, normalize.
  4. y = Wp @ out -> DMA out fp32.
"""

import numpy as np

B = 4
C = 512
HH = 48
WW = 48
N = HH * WW          # 2304
NQ = N // 2          # 1152 queries per core
HEADS = 8
D = C // HEADS       # 64
SCALE = float(D) ** -0.5
NCORES = 8

_CACHE: dict = {}


def _build_module(stage=4, npairs=4, nchunks=2, do_rem=True, rem_add=True, rem_av=True, loop_n=None):
    import concourse.mybir as mybir
    import concourse.tile as tile
    from concourse import bacc

    FP16 = mybir.dt.float16
    FP32 = mybir.dt.float32
    AF = mybir.ActivationFunctionType

    nc = bacc.Bacc("TRN2", target_bir_lowering=False, debug=False)
    xk = nc.dram_tensor("xk", [C, N], FP16, kind="ExternalInput")
    xq = nc.dram_tensor("xq", [C, NQ], FP16, kind="ExternalInput")
    wqkv = nc.dram_tensor("wqkv", [C, 3 * C], FP16, kind="ExternalInput")
    wproj = nc.dram_tensor("wproj", [C, C], FP16, kind="ExternalInput")
    y = nc.dram_tensor("y", [C, NQ], FP32, kind="ExternalOutput")

    CT = C // 128     # 4 channel tiles
    MT = N // 128     # 18 key tiles
    NQ32 = [(0, 512), (512, 512), (1024, 128)]

    with tile.TileContext(nc) as tc:
        with (
            tc.tile_pool(name="consts", bufs=1) as cpool,
            tc.tile_pool(name="wts", bufs=1) as wpool,
            tc.tile_pool(name="qkv", bufs=1) as qkpool,
            tc.tile_pool(name="keep", bufs=1) as keep,
        ):
            ones1 = cpool.tile([1, 64], FP16, name="ones1", tag="ones1")
            nc.vector.memset(ones1[:], 1.0)

            wt = []
            wp = []
            for t in range(CT):
                w = wpool.tile([128, 3 * C], FP16, name=f"w{t}", tag=f"w{t}")
                nc.sync.dma_start(w[:], wqkv.ap()[128 * t : 128 * (t + 1), :])
                wt.append(w)
                p = wpool.tile([128, C], FP16, name=f"wp{t}", tag=f"wp{t}")
                nc.sync.dma_start(p[:], wproj.ap()[128 * t : 128 * (t + 1), :])
                wp.append(p)

            qsb = [qkpool.tile([128, NQ], FP16, name=f"q{t}", tag=f"q{t}") for t in range(CT)]
            ksb = [qkpool.tile([128, N], FP16, name=f"k{t}", tag=f"k{t}") for t in range(CT)]
            vsb = [qkpool.tile([128, 520], FP16, name=f"v{m}", tag=f"v{m}") for m in range(MT)]

            def phase1():
                with (
                    tc.tile_pool(name="ph1", bufs=1) as ph1,
                    tc.tile_pool(name="p1ps", bufs=1, space="PSUM") as pps,
                    tc.tile_pool(name="p1psv", bufs=2, space="PSUM") as ppsv,
                ):
                    xf = []
                    xqt = []
                    for t in range(CT):
                        xt = ph1.tile([128, N], FP16, name=f"x{t}", tag=f"x{t}")
                        nc.sync.dma_start(xt[:], xk.ap()[128 * t : 128 * (t + 1), :])
                        xf.append(xt)
                        xs = ph1.tile([128, NQ], FP16, name=f"xq{t}", tag=f"xq{t}")
                        nc.sync.dma_start(xs[:], xq.ap()[128 * t : 128 * (t + 1), :])
                        xqt.append(xs)

                    # vT[n, cv] = sum_c xk[c, n] * WvT[c, cv]
                    for m in range(MT):
                        psv = ppsv.tile([128, C], FP32, name="psv", tag="psv")
                        for kt in range(CT):
                            nc.tensor.matmul(
                                psv[:, 0:512],
                                lhsT=xf[kt][:, 128 * m : 128 * (m + 1)],
                                rhs=wt[kt][:, 2 * C : 3 * C],
                                start=(kt == 0),
                                stop=(kt == CT - 1),
                            )
                        nc.vector.memset(vsb[m][:], 1.0)
                        nc.vector.tensor_copy(
                            vsb[m][:].rearrange("p (h w) -> p h w", h=8)[:, :, 0:64],
                            psv[:].rearrange("p (h w) -> p h w", h=8),
                        )

                    for ct in range(CT):
                        psq = pps.tile([128, NQ], FP32, name="psq", tag="psq")
                        for kt in range(CT):
                            for c0, cl in NQ32:
                                nc.tensor.matmul(
                                    psq[:, c0 : c0 + cl],
                                    lhsT=wt[kt][:, 128 * ct : 128 * (ct + 1)],
                                    rhs=xqt[kt][:, c0 : c0 + cl],
                                    start=(kt == 0),
                                    stop=(kt == CT - 1),
                                )
                        nc.vector.tensor_copy(qsb[ct][:], psq[:])
                        for n0, nl in [(0, 1024), (1024, 1024), (2048, 256)]:
                            psk = pps.tile([128, 1024], FP32, name="psk", tag="psk")
                            for kt in range(CT):
                                for c0 in range(0, nl, 512):
                                    cl = min(512, nl - c0)
                                    nc.tensor.matmul(
                                        psk[:, c0 : c0 + cl],
                                        lhsT=wt[kt][:, C + 128 * ct : C + 128 * (ct + 1)],
                                        rhs=xf[kt][:, n0 + c0 : n0 + c0 + cl],
                                        start=(kt == 0),
                                        stop=(kt == CT - 1),
                                    )
                            nc.vector.tensor_copy(ksb[ct][:, n0 : n0 + nl], psk[:, 0:nl])

            # avsb: partitions 0:64 head A dims, 64:128 head B; cols nq 0:1152.
            # cssb[t][half]: softmax denominators (fp32) for head pair t.
            def phase2(avsb, cssb, oa, oy):
                with (
                    tc.tile_pool(name="sps", bufs=2, space="PSUM") as spool,
                    tc.tile_pool(name="avps", bufs=1, space="PSUM") as avps,
                    tc.tile_pool(name="flex", bufs=2, space="PSUM") as flex,
                    tc.tile_pool(name="esb", bufs=4) as epool,
                    tc.tile_pool(name="p2sb", bufs=2) as p2sb,
                ):
                    for t in range(CT):
                        kA = ksb[t][0:64, :]
                        kB = ksb[t][64:128, :]
                        qA = qsb[t][0:64, :]
                        qB = qsb[t][64:128, :]
                        for c in range(nchunks):            # main nq chunks of 512
                            nq0 = 512 * c
                            avA = avps.tile([65, 512], FP32, name="avA", tag="avA")
                            avB = avps.tile([65, 512], FP32, name="avB", tag="avB")
                            for m in range(MT):
                                ms = slice(128 * m, 128 * (m + 1))
                                sp = spool.tile([128, 1024], FP32, name="s", tag="s")
                                nc.tensor.matmul(
                                    sp[:, 0:512], lhsT=kA[:, ms],
                                    rhs=qA[:, nq0 : nq0 + 512],
                                    start=True, stop=True, tile_position=(0, 0),
                                )
                                nc.tensor.matmul(
                                    sp[:, 512:1024], lhsT=kB[:, ms],
                                    rhs=qB[:, nq0 : nq0 + 512],
                                    start=True, stop=True, tile_position=(64, 0),
                                )
                                es = epool.tile([128, 1024], FP16, name="es", tag="es")
                                nc.scalar.activation(es[:], sp[:], AF.Exp, scale=SCALE)
                                nc.tensor.matmul(
                                    avA[:], lhsT=vsb[m][:, 130 * t : 130 * t + 65],
                                    rhs=es[:, 0:512],
                                    start=(m == 0), stop=(m == MT - 1),
                                )
                                nc.tensor.matmul(
                                    avB[:], lhsT=vsb[m][:, 130 * t + 65 : 130 * t + 130],
                                    rhs=es[:, 512:1024],
                                    start=(m == 0), stop=(m == MT - 1),
                                )
                            nc.vector.tensor_copy(avsb[t][0:64, nq0 : nq0 + 512], avA[0:64, :])
                            nc.vector.tensor_copy(avsb[t][64:128, nq0 : nq0 + 512], avB[0:64, :])
                            nc.vector.tensor_copy(cssb[t][0][:, nq0 : nq0 + 512], avA[64:65, :])
                            nc.vector.tensor_copy(cssb[t][1][:, nq0 : nq0 + 512], avB[64:65, :])
                        # remainder nq chunk (128 queries), exp batched over
                        # groups of 4 key tiles: bank0 = A rems, bank1 = B rems
                        avrA = avps.tile([65, 128], FP32, name="avrA", tag="avA")
                        avrB = avps.tile([65, 128], FP32, name="avrB", tag="avB")
                        for g0 in range(0, MT, 4):
                            gm = min(4, MT - g0)
                            sr = spool.tile([128, 1024], FP32, name="sr", tag="s")
                            for j in range(gm):
                                ms = slice(128 * (g0 + j), 128 * (g0 + j + 1))
                                nc.tensor.matmul(
                                    sr[:, 128 * j : 128 * j + 128], lhsT=kA[:, ms],
                                    rhs=qA[:, 1024:1152],
                                    start=True, stop=True, tile_position=(0, 0),
                                )
                                nc.tensor.matmul(
                                    sr[:, 512 + 128 * j : 512 + 128 * j + 128], lhsT=kB[:, ms],
                                    rhs=qB[:, 1024:1152],
                                    start=True, stop=True, tile_position=(64, 0),
                                )
                            er = epool.tile([128, 1024], FP16, name="er", tag="er", bufs=2)
                            if gm == 4:
                                nc.scalar.activation(er[:], sr[:], AF.Exp, scale=SCALE)
                            else:
                                nc.scalar.activation(
                                    er[:, 0 : 128 * gm], sr[:, 0 : 128 * gm],
                                    AF.Exp, scale=SCALE,
                                )
                                nc.scalar.activation(
                                    er[:, 512 : 512 + 128 * gm], sr[:, 512 : 512 + 128 * gm],
                                    AF.Exp, scale=SCALE,
                                )
                            for j in range(gm):
                                m = g0 + j
                                nc.tensor.matmul(
                                    avrA[:], lhsT=vsb[m][:, 130 * t : 130 * t + 65],
                                    rhs=er[:, 128 * j : 128 * j + 128],
                                    start=(m == 0), stop=(m == MT - 1),
                                )
                                nc.tensor.matmul(
                                    avrB[:], lhsT=vsb[m][:, 130 * t + 65 : 130 * t + 130],
                                    rhs=er[:, 512 + 128 * j : 512 + 128 * j + 128],
                                    start=(m == 0), stop=(m == MT - 1),
                                )
                        nc.vector.tensor_copy(avsb[t][0:64, 1024:1152], avrA[0:64, :])
                        nc.vector.tensor_copy(avsb[t][64:128, 1024:1152], avrB[0:64, :])
                        nc.vector.tensor_copy(cssb[t][0][:, 1024:1152], avrA[64:65, :])
                        nc.vector.tensor_copy(cssb[t][1][:, 1024:1152], avrB[64:65, :])
                        # normalize this pair (interleaved phase 3)
                        for half in range(2):
                            rec = p2sb.tile([1, NQ], FP16, name="rec", tag=f"rec{half}")
                            with nc.allow_low_precision(reason="softmax recip fp16"):
                                nc.vector.reciprocal(rec[:], cssb[t][half][:])
                            for c0, cl in NQ32:
                                bc = flex.tile([128, 512], FP32, name="bc", tag="flex")
                                nc.tensor.matmul(
                                    bc[0:64, 0:cl], lhsT=ones1[:],
                                    rhs=rec[:, c0 : c0 + cl],
                                    start=True, stop=True,
                                )
                                nc.vector.tensor_mul(
                                    oa[t][64 * half : 64 * half + 64, c0 : c0 + cl],
                                    avsb[t][64 * half : 64 * half + 64, c0 : c0 + cl],
                                    bc[0:64, 0:cl],
                                )
                        # output-projection partial for this pair:
                        # oy[ct] += WpT[t-block, ct-block].T @ oa[t]
                        for ct in range(CT):
                            for c0, cl in NQ32:
                                py = flex.tile([128, 512], FP32, name="py", tag="flex")
                                nc.tensor.matmul(
                                    py[:, 0:cl],
                                    lhsT=wp[t][:, 128 * ct : 128 * (ct + 1)],
                                    rhs=oa[t][:, c0 : c0 + cl],
                                    start=True, stop=True,
                                )
                                if t == 0:
                                    nc.vector.tensor_copy(
                                        oy[ct][:, c0 : c0 + cl], py[:, 0:cl]
                                    )
                                else:
                                    nc.vector.tensor_add(
                                        oy[ct][:, c0 : c0 + cl],
                                        oy[ct][:, c0 : c0 + cl],
                                        py[:, 0:cl],
                                    )

            def dump(tiles):
                with tc.tile_pool(name="dbg", bufs=2) as dbg:
                    for ct in range(len(tiles)):
                        yd = dbg.tile([128, NQ], FP32, name="yd", tag="yd")
                        nc.vector.tensor_copy(yd[:], tiles[ct][:])
                        nc.sync.dma_start(y.ap()[128 * ct : 128 * (ct + 1), :], yd[:])

            def _run_phases():
                phase1()
                if stage == 1:
                    dump(qsb)
                    return
                avsb = [keep.tile([128, NQ], FP16, name=f"av{t}", tag=f"av{t}") for t in range(CT)]
                cssb = [[keep.tile([1, NQ], FP32, name=f"cs{t}_{h}", tag=f"cs{t}_{h}") for h in range(2)] for t in range(CT)]
                oa = [keep.tile([128, NQ], FP16, name=f"oa{t}", tag=f"oa{t}") for t in range(CT)]
                oy = [keep.tile([128, NQ], FP32, name=f"oy{t}", tag=f"oy{t}") for t in range(CT)]
                phase2(avsb, cssb, oa, oy)
                if stage == 2:
                    dump(avsb[:npairs])
                    return
                if stage == 3:
                    dump(oa)
                    return
                for ct in range(CT):
                    nc.sync.dma_start(y.ap()[128 * ct : 128 * (ct + 1), :], oy[ct][:])

            import contextlib
            loop_ctx = tc.For_i(0, loop_n, 1) if loop_n else contextlib.nullcontext()
            with loop_ctx:
                _run_phases()

    nc.compile()
    return nc


def _get_module():
    if "nc" not in _CACHE:
        _CACHE["nc"] = _build_module()
    return _CACHE["nc"]


def make_in_maps(x, qkv_w, proj_w):
    xf = np.asarray(x, dtype=np.float32).reshape(B, C, N)
    wq = np.ascontiguousarray(np.asarray(qkv_w).T).astype(np.float16)
    wpj = np.ascontiguousarray(np.asarray(proj_w).T).astype(np.float16)
    in_maps = []
    for i in range(NCORES):
        b, h = divmod(i, 2)
        xkc = np.ascontiguousarray(xf[b]).astype(np.float16)
        xqc = np.ascontiguousarray(xf[b][:, h * NQ : (h + 1) * NQ]).astype(np.float16)
        in_maps.append({"xk": xkc, "xq": xqc, "wqkv": wq, "wproj": wpj})
    return in_maps


def gather_out(results):
    out = np.empty((B, C, N), np.float32)
    for i in range(NCORES):
        b, h = divmod(i, 2)
        out[b][:, h * NQ : (h + 1) * NQ] = results[i]["y"]
    return out.reshape(B, C, HH, WW)


def kernel(x, qkv_w, proj_w):
    from concourse import bass_utils

    nc = _get_module()
    in_maps = make_in_maps(x, qkv_w, proj_w)
    res = bass_utils.run_bass_kernel_spmd(
        nc, in_maps, core_ids=list(range(NCORES)), trace=False
    )
    return gather_out(res.results)

